# revision 34
# baseline (speedup 1.0000x reference)
"""Trainium2 Bass kernel for nn_CrossAttention_45286135169187.

Math (per batch b, with storage st [DIM, HW], tq = w_q*target + b_q [HW]):
    u[c]      = sum_x st[c,x] * tq[x]
    s         = sum_x tq[x]
    scores[k] = sum_c w_ca[DIM+k, c] * u[c] + b_ca[DIM+k] * s
    attn      = softmax(scores)
    vT[c]     = sum_k attn[k] * w_ca[k, c]
    beta      = sum_k attn[k] * b_ca[k]
    out[d, x] = sum_c vT[c] * st[c,x] + beta     (identical for all d)

The baseline implementation was DMA-bound: 38 MB/core of HBM traffic at
~369 GB/s -> ~103 us/iter. This version cuts traffic to the 25.2 MB/core
floor (16.8 MB f32 storage read + 8.4 MB bf16 output write) and runs at
~65 us/iter with the DMA rings >98% occupied:
  - storage is loaded ONCE, casting f32->bf16 in the SWDGE DMA datapath
    (no on-chip convert pass, half the SBUF footprint, deep prefetch);
    pass 1 (u) runs on the bf16 copy via DVE STT with an f32 accumulator.
  - tq is partition-broadcast by a PE rank-1 matmul (lhsT = w_q*ones row,
    rhs = bf16 target row) + ACT evacuation instead of a DRAM round trip;
    the evacuation accumulators also produce s for free.
  - the output is written as bf16 (rel-err gate is 2e-2; measured 3e-3 vs
    a CPU reference) and upcast to f32 on the host. The 512 identical
    output channels are written by broadcast-source DMAs (stride-0 source
    AP repeats the [128, HW] tile 4x), quarter-rows at a time.
  - beta rides as a 5th column of the vT matmul group and is fused into
    the pass-2 PSUM evacuation via ACT Identity's per-partition AP bias.
  - the scores path (u accumulate, wkT matmul, softmax) stays f32.
Sharding: data-parallel over batch, 2 batches per core on 8 cores; the
small conv weights are replicated (w_ca pre-split/transposed on host).
"""

import time

import numpy as np

import concourse.mybir as mybir
import concourse.tile as tile
from concourse import bacc, bass2jax
from concourse.bass import ts

N_CORES = 8
B = 16
DIM = 512
H = 64
W = 64
HW = H * W          # 4096
NB = B // N_CORES   # batches per core = 2
P = 128             # partitions
NCH = DIM // P      # c-chunks = 4
BLK = 512           # x-block (psum bank) size
NBLK = HW // BLK    # 8
F32 = mybir.dt.float32
BF16 = mybir.dt.bfloat16
AX_X = mybir.AxisListType.X
OP_MUL = mybir.AluOpType.mult
OP_ADD = mybir.AluOpType.add
ACT_EXP = mybir.ActivationFunctionType.Exp
ACT_COPY = mybir.ActivationFunctionType.Copy
ACT_IDENT = mybir.ActivationFunctionType.Identity


def _emit(ctx, tc, ins, out, n_iters=1):
    nc = tc.nc
    storage, target, wkT, wvb, bvb, bk, wq_col, bq_col, bq4096 = ins

    singles = ctx.enter_context(tc.tile_pool(name="singles", bufs=1))
    stb_pool = ctx.enter_context(tc.tile_pool(name="stb", bufs=7))
    tqb_pool = ctx.enter_context(tc.tile_pool(name="tqb", bufs=2))
    outt_pool = ctx.enter_context(tc.tile_pool(name="outt", bufs=2))
    trow_pool = ctx.enter_context(tc.tile_pool(name="trow", bufs=2))
    small_pool = ctx.enter_context(tc.tile_pool(name="small", bufs=2))
    ps_tq = ctx.enter_context(tc.tile_pool(name="ps_tq", bufs=2, space="PSUM"))
    ps_out = ctx.enter_context(tc.tile_pool(name="ps_out", bufs=3, space="PSUM"))
    ps_small = ctx.enter_context(tc.tile_pool(name="ps_small", bufs=1, space="PSUM"))

    # ---- replicated constants (loaded once; scalar=ACT HWDGE ring) ----
    wqc_sb = singles.tile([P, 1], F32)
    nc.scalar.dma_start(out=wqc_sb, in_=wq_col)
    bqc_sb = singles.tile([P, 1], F32)
    nc.scalar.dma_start(out=bqc_sb, in_=bq_col)
    bq4096_sb = singles.tile([1, 1], F32)
    nc.scalar.dma_start(out=bq4096_sb, in_=bq4096)
    bk_sb = singles.tile([1, DIM], F32)
    nc.scalar.dma_start(out=bk_sb, in_=bk)
    wkT_sb = singles.tile([P, NCH, DIM], F32)   # [p, c-chunk, k]
    nc.scalar.dma_start(out=wkT_sb, in_=wkT.rearrange("(j p) k -> p j k", p=P))
    wvb_sb = singles.tile([P, NCH, DIM], BF16)  # [p, k-chunk, c]
    nc.scalar.dma_start(out=wvb_sb, in_=wvb.rearrange("(i p) c -> p i c", p=P))
    bvb_sb = singles.tile([P, NCH, P], BF16)    # bv chunk cols bcast on free
    nc.scalar.dma_start(out=bvb_sb, in_=bvb)

    one_11 = singles.tile([1, 1], F32)          # rhs for row->column transposes
    nc.vector.memset(one_11, 1.0)
    ones_pp = singles.tile([P, P], BF16)        # for vT free-dim broadcast
    nc.vector.memset(ones_pp, 1.0)
    ones_row16 = singles.tile([1, P], BF16)
    nc.vector.memset(ones_row16, 1.0)
    wq_row16 = singles.tile([1, P], BF16)       # lhsT for the tq broadcast mm
    nc.vector.tensor_scalar_mul(
        out=wq_row16, in0=ones_row16, scalar1=wqc_sb[0:1, :]
    )
    scratch16 = singles.tile([P, HW], BF16)     # pass-1 elementwise sink

    for it in range(n_iters):
        for b in range(NB):
            # ---- per-batch input loads; storage arrives as bf16 via the
            # SWDGE cast-DMA (f32 HBM read, bf16 SBUF write) ----
            trow16 = trow_pool.tile([1, HW], BF16, tag="trow16")
            nc.gpsimd.dma_start(out=trow16, in_=target[b : b + 1, :])
            sth = []
            for h in range(2):
                tb = stb_pool.tile([P, 2, HW], BF16, tag="stb")
                nc.gpsimd.dma_start(
                    out=tb,
                    in_=storage[b, 2 * P * h : 2 * P * (h + 1), :].rearrange(
                        "(j p) x -> p j x", p=P
                    ),
                )
                sth.append(tb)
            stb = [sth[j // 2][:, j % 2, :] for j in range(NCH)]

            # ---- tq broadcast: psum[p, x] = w_q * target[x]  (PE rank-1,
            # bf16); ACT evacuates to bf16 SBUF + accumulates row sums ----
            tqb = tqb_pool.tile([P, HW], BF16, tag="tqb")
            sacc = small_pool.tile([P, NBLK], F32, tag="sacc")
            for blk in range(NBLK):
                pst = ps_tq.tile([P, BLK], F32, tag="pst")
                nc.tensor.matmul(
                    pst, lhsT=wq_row16, rhs=trow16[:, ts(blk, BLK)],
                    start=True, stop=True,
                )
                nc.scalar.activation(
                    out=tqb[:, ts(blk, BLK)], in_=pst, func=ACT_COPY,
                    accum_out=sacc[:, blk : blk + 1],
                )
            # s = sum(w_q*t) + HW*b_q  (bias joins via bq4096)
            ssum = small_pool.tile([1, 1], F32, tag="ssum")
            nc.vector.reduce_sum(out=ssum, in_=sacc[0:1, :], axis=AX_X)
            s_t = small_pool.tile([1, 1], F32, tag="s")
            nc.vector.tensor_scalar(
                out=s_t, in0=ssum, scalar1=bq4096_sb, scalar2=None, op0=OP_ADD,
            )

            # ---- pass 1: u[c] = <st[c,:], tq> (DVE STT; the accumulator
            # sums the f32 products before any bf16 rounding) ----
            u_t = small_pool.tile([P, NCH], F32, tag="u")
            for j in range(NCH):
                nc.vector.scalar_tensor_tensor(
                    out=scratch16, in0=tqb, scalar=bqc_sb, in1=stb[j],
                    op0=OP_ADD, op1=OP_MUL, accum_out=u_t[:, j : j + 1],
                )

            # ---- scores row [1, DIM] = u @ wkT + s*bk (PE, accumulated) ----
            pssc = ps_small.tile([1, DIM], F32, tag="scores")
            for j in range(NCH):
                nc.tensor.matmul(
                    pssc, lhsT=u_t[:, j : j + 1], rhs=wkT_sb[:, j, :],
                    start=(j == 0), stop=False,
                )
            nc.tensor.matmul(pssc, lhsT=s_t, rhs=bk_sb, start=False, stop=True)

            # ---- softmax on one partition ----
            negmax = small_pool.tile([1, 1], F32, tag="negmax")
            nc.vector.reduce_max(out=negmax, in_=pssc, axis=AX_X, negate=True)
            attn = small_pool.tile([1, DIM], F32, tag="attn")
            sumexp = small_pool.tile([1, 1], F32, tag="sumexp")
            nc.scalar.activation(
                out=attn, in_=pssc, func=ACT_EXP, bias=negmax, scale=1.0,
                accum_out=sumexp,
            )
            rsum = small_pool.tile([1, 1], F32, tag="rsum")
            nc.vector.reciprocal(out=rsum, in_=sumexp)
            nc.scalar.activation(out=attn, in_=attn, func=ACT_COPY, scale=rsum)

            # ---- attn row -> bf16 columns [P, NCH] (PE transpose) ----
            psat = ps_small.tile([P, NCH], F32, tag="attnT")
            for j in range(NCH):
                nc.tensor.matmul(
                    psat[:, j : j + 1], lhsT=attn[:, ts(j, P)], rhs=one_11,
                    start=True, stop=True,
                )
            attnT = small_pool.tile([P, NCH], BF16, tag="attnTs")
            nc.scalar.copy(out=attnT, in_=psat)

            # ---- vT[c] = sum_k wv[k,c]*attn[k]; col 4 = beta (PE) ----
            psvt = ps_small.tile([P, NCH + 1], F32, tag="vT")
            for j in range(NCH):
                for i in range(NCH):
                    nc.tensor.matmul(
                        psvt[:, j : j + 1],
                        lhsT=wvb_sb[:, i, ts(j, P)], rhs=attnT[:, i : i + 1],
                        start=(i == 0), stop=(i == NCH - 1),
                    )
            for i in range(NCH):
                nc.tensor.matmul(
                    psvt[:, NCH : NCH + 1],
                    lhsT=bvb_sb[:, i, :], rhs=attnT[:, i : i + 1],
                    start=(i == 0), stop=(i == NCH - 1),
                )
            vTb = small_pool.tile([P, NCH + 1], F32, tag="vTs")
            nc.scalar.copy(out=vTb, in_=psvt)

            # broadcast each vT column across 128 stationary columns (bf16)
            vbc = small_pool.tile([P, NCH, P], BF16, tag="vbc")
            for j in range(NCH):
                nc.vector.tensor_scalar_mul(
                    out=vbc[:, j, :], in0=ones_pp, scalar1=vTb[:, j : j + 1]
                )

            # ---- pass 2 (bf16): psum[d, x] = sum_c vT[c]*st[c,x]; the ACT
            # Identity evacuation fuses +beta ----
            ot = outt_pool.tile([P, 1, HW], BF16, tag="ot")
            for blk in range(NBLK):
                pso = ps_out.tile([P, BLK], F32, tag="pso")
                for j in range(NCH):
                    nc.tensor.matmul(
                        pso, lhsT=vbc[:, j, :], rhs=stb[j][:, ts(blk, BLK)],
                        start=(j == 0), stop=(j == NCH - 1),
                    )
                nc.scalar.activation(
                    out=ot[:, 0, ts(blk, BLK)], in_=pso, func=ACT_IDENT,
                    bias=vTb[:, NCH : NCH + 1],
                )

            # the 512 output channels are identical: one broadcast-source
            # store per half writes the tile 4x (HWDGE ring)
            HQ = HW // 4
            dst = out[b].rearrange("(r p) x -> p r x", p=P)
            for q in range(4):
                hh = slice(q * HQ, (q + 1) * HQ)
                nc.sync.dma_start(
                    out=dst[:, :, hh],
                    in_=ot[:, 0:1, hh].to_broadcast((P, NCH, HQ)),
                )


def _build_program(n_iters=1):
    nc = bacc.Bacc(
        "TRN2", target_bir_lowering=False, debug=False, num_devices=N_CORES
    )
    storage = nc.dram_tensor("storage", [NB, DIM, HW], F32, kind="ExternalInput")
    target = nc.dram_tensor("target", [NB, HW], F32, kind="ExternalInput")
    wkT = nc.dram_tensor("wkT", [DIM, DIM], F32, kind="ExternalInput")
    wvb = nc.dram_tensor("wvb", [DIM, DIM], BF16, kind="ExternalInput")
    bvb = nc.dram_tensor("bvb", [P, NCH, P], BF16, kind="ExternalInput")
    bk = nc.dram_tensor("bk", [1, DIM], F32, kind="ExternalInput")
    wq = nc.dram_tensor("wq_col", [P, 1], F32, kind="ExternalInput")
    bq_col = nc.dram_tensor("bq_col", [P, 1], F32, kind="ExternalInput")
    bq4096 = nc.dram_tensor("bq4096", [1, 1], F32, kind="ExternalInput")
    out = nc.dram_tensor("out", [NB, DIM, HW], BF16, kind="ExternalOutput")

    from contextlib import ExitStack

    with tile.TileContext(nc) as tc, ExitStack() as ctx:
        _emit(
            ctx,
            tc,
            (
                storage.ap(), target.ap(), wkT.ap(), wvb.ap(), bvb.ap(),
                bk.ap(), wq.ap(), bq_col.ap(), bq4096.ap(),
            ),
            out.ap(),
            n_iters=n_iters,
        )
    nc.compile()
    return nc


class _Runner:
    """Jit-once PJRT executor for the compiled Bacc program (8-core SPMD)."""

    def __init__(self, nc):
        import jax
        from jax.experimental.shard_map import shard_map
        from jax.sharding import Mesh, PartitionSpec

        bass2jax.install_neuronx_cc_hook()
        self.jax = jax
        self.nc = nc
        partition_name = (
            nc.partition_id_tensor.name if nc.partition_id_tensor else None
        )
        in_names, out_names, out_avals, zero_outs = [], [], [], []
        for alloc in nc.m.functions[0].allocations:
            if not isinstance(alloc, mybir.MemoryLocationSet):
                continue
            name = alloc.memorylocations[0].name
            if alloc.kind == "ExternalInput":
                if name != partition_name:
                    in_names.append(name)
            elif alloc.kind == "ExternalOutput":
                shape = tuple(alloc.tensor_shape)
                dtype = mybir.dt.np(alloc.dtype)
                out_names.append(name)
                out_avals.append(jax.core.ShapedArray(shape, dtype))
                zero_outs.append(np.zeros(shape, dtype))
        self.in_names, self.out_names = in_names, out_names
        self.n_params = len(in_names)
        n_outs = len(out_avals)

        def _exec(params, out_bufs):
            ops = list(params) + list(out_bufs)
            if partition_name is not None:
                ops.append(bass2jax.partition_id_tensor())
            all_names = tuple(in_names) + tuple(out_names) + (
                (partition_name,) if partition_name else ()
            )
            return bass2jax._bass_exec_p.bind(
                *ops,
                out_avals=tuple(out_avals),
                in_names=all_names,
                out_names=tuple(out_names),
                lowering_input_output_aliases=(),
                sim_require_finite=True,
                sim_require_nnan=True,
                nc=nc,
            )

        def _body(*args):
            return tuple(_exec(args[: self.n_params], args[self.n_params :]))

        devices = jax.devices()[:N_CORES]
        self.mesh = Mesh(np.asarray(devices), ("core",))
        in_specs = (PartitionSpec("core"),) * (self.n_params + n_outs)
        out_specs = (PartitionSpec("core"),) * n_outs
        self.fn = jax.jit(
            shard_map(
                _body, mesh=self.mesh, in_specs=in_specs,
                out_specs=out_specs, check_rep=False,
            ),
            keep_unused=True,
        )
        self.zero_outs = zero_outs
        self._spec = PartitionSpec("core")

    def put_inputs(self, in_maps):
        import jax

        per_core = [[np.asarray(m[n]) for n in self.in_names] for m in in_maps]
        args = [
            np.concatenate([per_core[c][i] for c in range(N_CORES)], axis=0)
            for i in range(self.n_params)
        ]
        args += [np.concatenate([z] * N_CORES, axis=0) for z in self.zero_outs]
        sharding = jax.sharding.NamedSharding(self.mesh, self._spec)
        return [jax.device_put(a, sharding) for a in args]

    def run(self, dev_args):
        outs = self.fn(*dev_args)
        self.jax.block_until_ready(outs)
        return outs

    def results(self, outs):
        res = []
        for c in range(N_CORES):
            d = {}
            for i, name in enumerate(self.out_names):
                arr = np.asarray(outs[i])
                per = arr.shape[0] // N_CORES
                d[name] = arr[c * per : (c + 1) * per]
            res.append(d)
        return res


_CACHE = {}


def _get_runner(n_iters=1):
    key = n_iters
    if key not in _CACHE:
        _CACHE[key] = _Runner(_build_program(n_iters=n_iters))
    return _CACHE[key]


def _make_in_maps(storage, target, w_ca, b_ca, w_q, b_q):
    import ml_dtypes

    storage = np.asarray(storage, dtype=np.float32)
    target = np.asarray(target, dtype=np.float32)
    w_ca = np.asarray(w_ca, dtype=np.float32)
    b_ca = np.asarray(b_ca, dtype=np.float32)
    w_q = np.asarray(w_q, dtype=np.float32)
    b_q = np.asarray(b_q, dtype=np.float32)

    # host-side weight prep (tiny): split conv weight into V/K halves,
    # transpose the K half so the contraction dim lands on partitions
    wvb = np.ascontiguousarray(w_ca[:DIM]).astype(ml_dtypes.bfloat16)  # [k, c]
    wkT = np.ascontiguousarray(w_ca[DIM:].T)                           # [c, k]
    bv = b_ca[:DIM]
    # bvb[p, i, m] = bv[i*128 + p]  (chunk columns broadcast along free)
    bvb = np.broadcast_to(
        bv.reshape(NCH, P).T[:, :, None], (P, NCH, P)
    ).astype(ml_dtypes.bfloat16)
    bvb = np.ascontiguousarray(bvb)
    bk = b_ca[DIM:].reshape(1, DIM)
    wq_col = np.full((P, 1), w_q[0, 0], dtype=np.float32)
    bq_col = np.full((P, 1), b_q[0], dtype=np.float32)
    bq4096 = np.array([[b_q[0] * HW]], dtype=np.float32)

    st_flat = storage.reshape(B, DIM, HW)
    tg_flat = target.reshape(B, HW)
    in_maps = []
    for c in range(N_CORES):
        in_maps.append(
            {
                "storage": st_flat[c * NB : (c + 1) * NB],
                "target": tg_flat[c * NB : (c + 1) * NB],
                "wkT": wkT,
                "wvb": wvb,
                "bvb": bvb,
                "bk": bk,
                "wq_col": wq_col,
                "bq_col": bq_col,
                "bq4096": bq4096,
            }
        )
    return in_maps


def kernel(storage, target, w_ca, b_ca, w_q, b_q):
    runner = _get_runner()
    in_maps = _make_in_maps(storage, target, w_ca, b_ca, w_q, b_q)
    dev_args = runner.put_inputs(in_maps)
    outs = runner.run(dev_args)
    res = runner.results(outs)
    full = np.concatenate([r["out"] for r in res], axis=0)  # [B, DIM, HW] bf16
    return full.astype(np.float32).reshape(B, DIM, H, W)


def time_kernel(storage, target, w_ca, b_ca, w_q, b_q, n_iters=33, reps=8):
    """Estimate per-execution HW time from chained-NEFF wall-clock slope.
    NOTE: wall clock through the axon tunnel is noisy; prefer the NTFF
    trace numbers printed by test.py."""
    in_maps = _make_in_maps(storage, target, w_ca, b_ca, w_q, b_q)

    def best(runner):
        dev_args = runner.put_inputs(in_maps)
        runner.run(dev_args)  # warm the executable
        times = []
        for _ in range(reps):
            t0 = time.perf_counter()
            runner.run(dev_args)
            times.append(time.perf_counter() - t0)
        return min(times)

    t1 = best(_get_runner(1))
    tn = best(_get_runner(n_iters))
    per_exec = (tn - t1) / (n_iters - 1)
    return per_exec, t1, tn


# revision 37
# speedup vs baseline: 1.0286x; 1.0286x over previous
"""Trainium2 Bass kernel for nn_CrossAttention_45286135169187.

Math (per batch b, with storage st [DIM, HW], tq = w_q*target + b_q [HW]):
    u[c]      = sum_x st[c,x] * tq[x]
    s         = sum_x tq[x]
    scores[k] = sum_c w_ca[DIM+k, c] * u[c] + b_ca[DIM+k] * s
    attn      = softmax(scores)
    vT[c]     = sum_k attn[k] * w_ca[k, c]
    beta      = sum_k attn[k] * b_ca[k]
    out[d, x] = sum_c vT[c] * st[c,x] + beta     (identical for all d)

The baseline implementation was DMA-bound: 38 MB/core of HBM traffic at
~369 GB/s -> ~103 us/iter. This version cuts traffic to the 25.2 MB/core
floor (16.8 MB f32 storage read + 8.4 MB bf16 output write) and runs at
~65 us/iter with the DMA rings >98% occupied:
  - storage is loaded ONCE, casting f32->bf16 in the SWDGE DMA datapath
    (no on-chip convert pass, half the SBUF footprint, deep prefetch);
    pass 1 (u) runs on the bf16 copy via DVE STT with an f32 accumulator.
  - tq is partition-broadcast by a PE rank-1 matmul (lhsT = w_q*ones row,
    rhs = bf16 target row) + ACT evacuation instead of a DRAM round trip;
    the evacuation accumulators also produce s for free.
  - the output is written as bf16 (rel-err gate is 2e-2; measured 3e-3 vs
    a CPU reference) and upcast to f32 on the host. The 512 identical
    output channels are written as four plain full-row stores of the same
    [128, HW] tile (8 KB contiguous runs beat broadcast-source APs, whose
    4x smaller descriptors cost ~1.8x more SDMA engine time).
  - beta rides as a 5th column of the vT matmul group and is fused into
    the pass-2 PSUM evacuation via ACT Identity's per-partition AP bias.
  - the scores path (u accumulate, wkT matmul, softmax) stays f32.
Sharding: data-parallel over batch, 2 batches per core on 8 cores; the
small conv weights are replicated (w_ca pre-split/transposed on host).
"""

import time

import numpy as np

import concourse.mybir as mybir
import concourse.tile as tile
from concourse import bacc, bass2jax
from concourse.bass import ts

N_CORES = 8
B = 16
DIM = 512
H = 64
W = 64
HW = H * W          # 4096
NB = B // N_CORES   # batches per core = 2
P = 128             # partitions
NCH = DIM // P      # c-chunks = 4
BLK = 512           # x-block (psum bank) size
NBLK = HW // BLK    # 8
F32 = mybir.dt.float32
BF16 = mybir.dt.bfloat16
AX_X = mybir.AxisListType.X
OP_MUL = mybir.AluOpType.mult
OP_ADD = mybir.AluOpType.add
ACT_EXP = mybir.ActivationFunctionType.Exp
ACT_COPY = mybir.ActivationFunctionType.Copy
ACT_IDENT = mybir.ActivationFunctionType.Identity


def _emit(ctx, tc, ins, out, n_iters=1):
    nc = tc.nc
    storage, target, wkT, wvb, bvb, bk, wq_col, bq_col, bq4096 = ins

    singles = ctx.enter_context(tc.tile_pool(name="singles", bufs=1))
    stb_pool = ctx.enter_context(tc.tile_pool(name="stb", bufs=7))
    tqb_pool = ctx.enter_context(tc.tile_pool(name="tqb", bufs=2))
    outt_pool = ctx.enter_context(tc.tile_pool(name="outt", bufs=2))
    trow_pool = ctx.enter_context(tc.tile_pool(name="trow", bufs=2))
    small_pool = ctx.enter_context(tc.tile_pool(name="small", bufs=2))
    ps_tq = ctx.enter_context(tc.tile_pool(name="ps_tq", bufs=2, space="PSUM"))
    ps_out = ctx.enter_context(tc.tile_pool(name="ps_out", bufs=3, space="PSUM"))
    ps_small = ctx.enter_context(tc.tile_pool(name="ps_small", bufs=1, space="PSUM"))

    # ---- replicated constants (loaded once; scalar=ACT HWDGE ring) ----
    wqc_sb = singles.tile([P, 1], F32)
    nc.scalar.dma_start(out=wqc_sb, in_=wq_col)
    bqc_sb = singles.tile([P, 1], F32)
    nc.scalar.dma_start(out=bqc_sb, in_=bq_col)
    bq4096_sb = singles.tile([1, 1], F32)
    nc.scalar.dma_start(out=bq4096_sb, in_=bq4096)
    bk_sb = singles.tile([1, DIM], F32)
    nc.scalar.dma_start(out=bk_sb, in_=bk)
    wkT_sb = singles.tile([P, NCH, DIM], F32)   # [p, c-chunk, k]
    nc.scalar.dma_start(out=wkT_sb, in_=wkT.rearrange("(j p) k -> p j k", p=P))
    wvb_sb = singles.tile([P, NCH, DIM], BF16)  # [p, k-chunk, c]
    nc.scalar.dma_start(out=wvb_sb, in_=wvb.rearrange("(i p) c -> p i c", p=P))
    bvb_sb = singles.tile([P, NCH, P], BF16)    # bv chunk cols bcast on free
    nc.scalar.dma_start(out=bvb_sb, in_=bvb)

    one_11 = singles.tile([1, 1], F32)          # rhs for row->column transposes
    nc.vector.memset(one_11, 1.0)
    ones_pp = singles.tile([P, P], BF16)        # for vT free-dim broadcast
    nc.vector.memset(ones_pp, 1.0)
    ones_row16 = singles.tile([1, P], BF16)
    nc.vector.memset(ones_row16, 1.0)
    wq_row16 = singles.tile([1, P], BF16)       # lhsT for the tq broadcast mm
    nc.vector.tensor_scalar_mul(
        out=wq_row16, in0=ones_row16, scalar1=wqc_sb[0:1, :]
    )
    scratch16 = singles.tile([P, HW], BF16)     # pass-1 elementwise sink

    for it in range(n_iters):
        for b in range(NB):
            # ---- per-batch input loads; storage arrives as bf16 via the
            # SWDGE cast-DMA (f32 HBM read, bf16 SBUF write) ----
            trow16 = trow_pool.tile([1, HW], BF16, tag="trow16")
            nc.gpsimd.dma_start(out=trow16, in_=target[b : b + 1, :])
            sth = []
            for h in range(2):
                tb = stb_pool.tile([P, 2, HW], BF16, tag="stb")
                nc.gpsimd.dma_start(
                    out=tb,
                    in_=storage[b, 2 * P * h : 2 * P * (h + 1), :].rearrange(
                        "(j p) x -> p j x", p=P
                    ),
                )
                sth.append(tb)
            stb = [sth[j // 2][:, j % 2, :] for j in range(NCH)]

            # ---- tq broadcast: psum[p, x] = w_q * target[x]  (PE rank-1,
            # bf16); ACT evacuates to bf16 SBUF + accumulates row sums ----
            tqb = tqb_pool.tile([P, HW], BF16, tag="tqb")
            sacc = small_pool.tile([P, NBLK], F32, tag="sacc")
            for blk in range(NBLK):
                pst = ps_tq.tile([P, BLK], F32, tag="pst")
                nc.tensor.matmul(
                    pst, lhsT=wq_row16, rhs=trow16[:, ts(blk, BLK)],
                    start=True, stop=True,
                )
                nc.scalar.activation(
                    out=tqb[:, ts(blk, BLK)], in_=pst, func=ACT_COPY,
                    accum_out=sacc[:, blk : blk + 1],
                )
            # s = sum(w_q*t) + HW*b_q  (bias joins via bq4096)
            ssum = small_pool.tile([1, 1], F32, tag="ssum")
            nc.vector.reduce_sum(out=ssum, in_=sacc[0:1, :], axis=AX_X)
            s_t = small_pool.tile([1, 1], F32, tag="s")
            nc.vector.tensor_scalar(
                out=s_t, in0=ssum, scalar1=bq4096_sb, scalar2=None, op0=OP_ADD,
            )

            # ---- pass 1: u[c] = <st[c,:], tq> (DVE STT; the accumulator
            # sums the f32 products before any bf16 rounding) ----
            u_t = small_pool.tile([P, NCH], F32, tag="u")
            for j in range(NCH):
                nc.vector.scalar_tensor_tensor(
                    out=scratch16, in0=tqb, scalar=bqc_sb, in1=stb[j],
                    op0=OP_ADD, op1=OP_MUL, accum_out=u_t[:, j : j + 1],
                )

            # ---- scores row [1, DIM] = u @ wkT + s*bk (PE, accumulated) ----
            pssc = ps_small.tile([1, DIM], F32, tag="scores")
            for j in range(NCH):
                nc.tensor.matmul(
                    pssc, lhsT=u_t[:, j : j + 1], rhs=wkT_sb[:, j, :],
                    start=(j == 0), stop=False,
                )
            nc.tensor.matmul(pssc, lhsT=s_t, rhs=bk_sb, start=False, stop=True)

            # ---- softmax on one partition ----
            negmax = small_pool.tile([1, 1], F32, tag="negmax")
            nc.vector.reduce_max(out=negmax, in_=pssc, axis=AX_X, negate=True)
            attn = small_pool.tile([1, DIM], F32, tag="attn")
            sumexp = small_pool.tile([1, 1], F32, tag="sumexp")
            nc.scalar.activation(
                out=attn, in_=pssc, func=ACT_EXP, bias=negmax, scale=1.0,
                accum_out=sumexp,
            )
            rsum = small_pool.tile([1, 1], F32, tag="rsum")
            nc.vector.reciprocal(out=rsum, in_=sumexp)
            nc.scalar.activation(out=attn, in_=attn, func=ACT_COPY, scale=rsum)

            # ---- attn row -> bf16 columns [P, NCH] (PE transpose) ----
            psat = ps_small.tile([P, NCH], F32, tag="attnT")
            for j in range(NCH):
                nc.tensor.matmul(
                    psat[:, j : j + 1], lhsT=attn[:, ts(j, P)], rhs=one_11,
                    start=True, stop=True,
                )
            attnT = small_pool.tile([P, NCH], BF16, tag="attnTs")
            nc.scalar.copy(out=attnT, in_=psat)

            # ---- vT[c] = sum_k wv[k,c]*attn[k]; col 4 = beta (PE) ----
            psvt = ps_small.tile([P, NCH + 1], F32, tag="vT")
            for j in range(NCH):
                for i in range(NCH):
                    nc.tensor.matmul(
                        psvt[:, j : j + 1],
                        lhsT=wvb_sb[:, i, ts(j, P)], rhs=attnT[:, i : i + 1],
                        start=(i == 0), stop=(i == NCH - 1),
                    )
            for i in range(NCH):
                nc.tensor.matmul(
                    psvt[:, NCH : NCH + 1],
                    lhsT=bvb_sb[:, i, :], rhs=attnT[:, i : i + 1],
                    start=(i == 0), stop=(i == NCH - 1),
                )
            vTb = small_pool.tile([P, NCH + 1], F32, tag="vTs")
            nc.scalar.copy(out=vTb, in_=psvt)

            # broadcast each vT column across 128 stationary columns (bf16)
            vbc = small_pool.tile([P, NCH, P], BF16, tag="vbc")
            for j in range(NCH):
                nc.vector.tensor_scalar_mul(
                    out=vbc[:, j, :], in0=ones_pp, scalar1=vTb[:, j : j + 1]
                )

            # ---- pass 2 (bf16): psum[d, x] = sum_c vT[c]*st[c,x]; the ACT
            # Identity evacuation fuses +beta ----
            ot = outt_pool.tile([P, 1, HW], BF16, tag="ot")
            for blk in range(NBLK):
                pso = ps_out.tile([P, BLK], F32, tag="pso")
                for j in range(NCH):
                    nc.tensor.matmul(
                        pso, lhsT=vbc[:, j, :], rhs=stb[j][:, ts(blk, BLK)],
                        start=(j == 0), stop=(j == NCH - 1),
                    )
                nc.scalar.activation(
                    out=ot[:, 0, ts(blk, BLK)], in_=pso, func=ACT_IDENT,
                    bias=vTb[:, NCH : NCH + 1],
                )

            # the 512 output channels are identical: write the tile 4x with
            # plain 2-D stores (8 KB contiguous per partition, HWDGE ring)
            for dj in range(NCH):
                nc.sync.dma_start(
                    out=out[b, ts(dj, P), :], in_=ot[:, 0, :],
                )


def _build_program(n_iters=1):
    nc = bacc.Bacc(
        "TRN2", target_bir_lowering=False, debug=False, num_devices=N_CORES
    )
    storage = nc.dram_tensor("storage", [NB, DIM, HW], F32, kind="ExternalInput")
    target = nc.dram_tensor("target", [NB, HW], F32, kind="ExternalInput")
    wkT = nc.dram_tensor("wkT", [DIM, DIM], F32, kind="ExternalInput")
    wvb = nc.dram_tensor("wvb", [DIM, DIM], BF16, kind="ExternalInput")
    bvb = nc.dram_tensor("bvb", [P, NCH, P], BF16, kind="ExternalInput")
    bk = nc.dram_tensor("bk", [1, DIM], F32, kind="ExternalInput")
    wq = nc.dram_tensor("wq_col", [P, 1], F32, kind="ExternalInput")
    bq_col = nc.dram_tensor("bq_col", [P, 1], F32, kind="ExternalInput")
    bq4096 = nc.dram_tensor("bq4096", [1, 1], F32, kind="ExternalInput")
    out = nc.dram_tensor("out", [NB, DIM, HW], BF16, kind="ExternalOutput")

    from contextlib import ExitStack

    with tile.TileContext(nc) as tc, ExitStack() as ctx:
        _emit(
            ctx,
            tc,
            (
                storage.ap(), target.ap(), wkT.ap(), wvb.ap(), bvb.ap(),
                bk.ap(), wq.ap(), bq_col.ap(), bq4096.ap(),
            ),
            out.ap(),
            n_iters=n_iters,
        )
    nc.compile()
    return nc


class _Runner:
    """Jit-once PJRT executor for the compiled Bacc program (8-core SPMD)."""

    def __init__(self, nc):
        import jax
        from jax.experimental.shard_map import shard_map
        from jax.sharding import Mesh, PartitionSpec

        bass2jax.install_neuronx_cc_hook()
        self.jax = jax
        self.nc = nc
        partition_name = (
            nc.partition_id_tensor.name if nc.partition_id_tensor else None
        )
        in_names, out_names, out_avals, zero_outs = [], [], [], []
        for alloc in nc.m.functions[0].allocations:
            if not isinstance(alloc, mybir.MemoryLocationSet):
                continue
            name = alloc.memorylocations[0].name
            if alloc.kind == "ExternalInput":
                if name != partition_name:
                    in_names.append(name)
            elif alloc.kind == "ExternalOutput":
                shape = tuple(alloc.tensor_shape)
                dtype = mybir.dt.np(alloc.dtype)
                out_names.append(name)
                out_avals.append(jax.core.ShapedArray(shape, dtype))
                zero_outs.append(np.zeros(shape, dtype))
        self.in_names, self.out_names = in_names, out_names
        self.n_params = len(in_names)
        n_outs = len(out_avals)

        def _exec(params, out_bufs):
            ops = list(params) + list(out_bufs)
            if partition_name is not None:
                ops.append(bass2jax.partition_id_tensor())
            all_names = tuple(in_names) + tuple(out_names) + (
                (partition_name,) if partition_name else ()
            )
            return bass2jax._bass_exec_p.bind(
                *ops,
                out_avals=tuple(out_avals),
                in_names=all_names,
                out_names=tuple(out_names),
                lowering_input_output_aliases=(),
                sim_require_finite=True,
                sim_require_nnan=True,
                nc=nc,
            )

        def _body(*args):
            return tuple(_exec(args[: self.n_params], args[self.n_params :]))

        devices = jax.devices()[:N_CORES]
        self.mesh = Mesh(np.asarray(devices), ("core",))
        in_specs = (PartitionSpec("core"),) * (self.n_params + n_outs)
        out_specs = (PartitionSpec("core"),) * n_outs
        self.fn = jax.jit(
            shard_map(
                _body, mesh=self.mesh, in_specs=in_specs,
                out_specs=out_specs, check_rep=False,
            ),
            keep_unused=True,
        )
        self.zero_outs = zero_outs
        self._spec = PartitionSpec("core")

    def put_inputs(self, in_maps):
        import jax

        per_core = [[np.asarray(m[n]) for n in self.in_names] for m in in_maps]
        args = [
            np.concatenate([per_core[c][i] for c in range(N_CORES)], axis=0)
            for i in range(self.n_params)
        ]
        args += [np.concatenate([z] * N_CORES, axis=0) for z in self.zero_outs]
        sharding = jax.sharding.NamedSharding(self.mesh, self._spec)
        return [jax.device_put(a, sharding) for a in args]

    def run(self, dev_args):
        outs = self.fn(*dev_args)
        self.jax.block_until_ready(outs)
        return outs

    def results(self, outs):
        res = []
        for c in range(N_CORES):
            d = {}
            for i, name in enumerate(self.out_names):
                arr = np.asarray(outs[i])
                per = arr.shape[0] // N_CORES
                d[name] = arr[c * per : (c + 1) * per]
            res.append(d)
        return res


_CACHE = {}


def _get_runner(n_iters=1):
    key = n_iters
    if key not in _CACHE:
        _CACHE[key] = _Runner(_build_program(n_iters=n_iters))
    return _CACHE[key]


def _make_in_maps(storage, target, w_ca, b_ca, w_q, b_q):
    import ml_dtypes

    storage = np.asarray(storage, dtype=np.float32)
    target = np.asarray(target, dtype=np.float32)
    w_ca = np.asarray(w_ca, dtype=np.float32)
    b_ca = np.asarray(b_ca, dtype=np.float32)
    w_q = np.asarray(w_q, dtype=np.float32)
    b_q = np.asarray(b_q, dtype=np.float32)

    # host-side weight prep (tiny): split conv weight into V/K halves,
    # transpose the K half so the contraction dim lands on partitions
    wvb = np.ascontiguousarray(w_ca[:DIM]).astype(ml_dtypes.bfloat16)  # [k, c]
    wkT = np.ascontiguousarray(w_ca[DIM:].T)                           # [c, k]
    bv = b_ca[:DIM]
    # bvb[p, i, m] = bv[i*128 + p]  (chunk columns broadcast along free)
    bvb = np.broadcast_to(
        bv.reshape(NCH, P).T[:, :, None], (P, NCH, P)
    ).astype(ml_dtypes.bfloat16)
    bvb = np.ascontiguousarray(bvb)
    bk = b_ca[DIM:].reshape(1, DIM)
    wq_col = np.full((P, 1), w_q[0, 0], dtype=np.float32)
    bq_col = np.full((P, 1), b_q[0], dtype=np.float32)
    bq4096 = np.array([[b_q[0] * HW]], dtype=np.float32)

    st_flat = storage.reshape(B, DIM, HW)
    tg_flat = target.reshape(B, HW)
    in_maps = []
    for c in range(N_CORES):
        in_maps.append(
            {
                "storage": st_flat[c * NB : (c + 1) * NB],
                "target": tg_flat[c * NB : (c + 1) * NB],
                "wkT": wkT,
                "wvb": wvb,
                "bvb": bvb,
                "bk": bk,
                "wq_col": wq_col,
                "bq_col": bq_col,
                "bq4096": bq4096,
            }
        )
    return in_maps


def kernel(storage, target, w_ca, b_ca, w_q, b_q):
    runner = _get_runner()
    in_maps = _make_in_maps(storage, target, w_ca, b_ca, w_q, b_q)
    dev_args = runner.put_inputs(in_maps)
    outs = runner.run(dev_args)
    res = runner.results(outs)
    full = np.concatenate([r["out"] for r in res], axis=0)  # [B, DIM, HW] bf16
    return full.astype(np.float32).reshape(B, DIM, H, W)


def time_kernel(storage, target, w_ca, b_ca, w_q, b_q, n_iters=33, reps=8):
    """Estimate per-execution HW time from chained-NEFF wall-clock slope.
    NOTE: wall clock through the axon tunnel is noisy; prefer the NTFF
    trace numbers printed by test.py."""
    in_maps = _make_in_maps(storage, target, w_ca, b_ca, w_q, b_q)

    def best(runner):
        dev_args = runner.put_inputs(in_maps)
        runner.run(dev_args)  # warm the executable
        times = []
        for _ in range(reps):
            t0 = time.perf_counter()
            runner.run(dev_args)
            times.append(time.perf_counter() - t0)
        return min(times)

    t1 = best(_get_runner(1))
    tn = best(_get_runner(n_iters))
    per_exec = (tn - t1) / (n_iters - 1)
    return per_exec, t1, tn
